# revision 11
# baseline (speedup 1.0000x reference)
"""Trainium2 Bass kernel for nn_ContextPooling (conv stack -> LN -> softmax ->
Gaussian attention pooling). Data-parallel over batch across 8 NeuronCores.

Math notes:
  - conv1d(k=3, same) is computed as 3 shifted matmuls accumulated in PSUM.
  - LayerNorm over (C, L) jointly: stats via activation accum_out (sum) +
    Square pass (sumsq), cross-partition reduction via ones-matmul on PE.
  - gauss / max(gauss, axis=j) == exp(-(i-j)^2 / (1e-5 + 2*std_i^2)) exactly
    (the c/std prefactor is constant along j and cancels).
  - att[i, j] = exp(-(i-j)^2 * a_i) * w_j, with w_j folded in as a per-
    partition bias ln(w_j) inside the ACT Exp.
  - att output: out[i, d] = sum_j attT[j, i] * feats[j, d] -> matmul with
    lhsT = attT tiles (j on partitions), rhs = feats (natural layout).
"""
import numpy as np
import ml_dtypes

import concourse.bass as bass
import concourse.tile as tile
from concourse import mybir

F32 = mybir.dt.float32
BF16 = mybir.dt.bfloat16
AF = mybir.ActivationFunctionType
ALU = mybir.AluOpType
AX = mybir.AxisListType

B, L, D, D2 = 16, 1024, 768, 1536
N_CORES = 8
KW = 3
R = 0.1
BF = ml_dtypes.bfloat16


def emit_context_pooling(tc, out_ap, ins, *, BPC, L, D, D2):
    nc = tc.nc
    DT, CT, JT = D // 128, D2 // 128, L // 128
    NSPL = min(512, L)
    NH = L // NSPL
    RL = R * L
    att_splits = [(n0, min(512, D - n0)) for n0 in range(0, D, 512)]
    l_splits = [(n0, min(512, L - n0)) for n0 in range(0, L, 512)]

    with (
        tc.tile_pool(name="const", bufs=1) as cpool,
        tc.tile_pool(name="sb", bufs=1) as sb,
        tc.tile_pool(name="ps", bufs=1, space="PSUM") as pp,
    ):
        # ---------------- constants ----------------
        ones_col = cpool.tile([128, 1], F32)
        nc.gpsimd.memset(ones_col, 1.0)
        ones_row = cpool.tile([1, 128], F32)
        nc.gpsimd.memset(ones_row, 1.0)
        negones_row = cpool.tile([1, 128], F32)
        nc.gpsimd.memset(negones_row, -1.0)
        e1col = cpool.tile([2, 1], F32)  # selector (0, 1)^T
        nc.sync.dma_start(e1col, ins["e1p"])
        w3sb = cpool.tile([128, CT, KW, 2], BF16)
        nc.sync.dma_start(w3sb, ins["w3p"])
        b1sb = cpool.tile([128, CT], F32)
        nc.sync.dma_start(b1sb, ins["b1p"])
        b2sb = cpool.tile([128, CT], F32)
        nc.sync.dma_start(b2sb, ins["b2p"])
        b3sb = cpool.tile([2, 1], F32)
        nc.sync.dma_start(b3sb, ins["b3p"])
        eps11 = cpool.tile([1, 1], F32)
        nc.gpsimd.memset(eps11, 1e-5)

        def ln_finalize(stats, p_src, nseg, ntot, p_out, name):
            """stats [p_src, 2*nseg] (sums | sumsqs) -> bc [p_out, 2] =
            (istd, -mean*istd) broadcast to p_out partitions."""
            pst = pp.tile([1, 2 * nseg], F32, tag="psmall", bufs=2, name=f"pst{name}")
            nc.tensor.matmul(pst, lhsT=ones_col[0:p_src, :], rhs=stats,
                             start=True, stop=True)
            stf = sb.tile([1, 2 * nseg], F32, tag="statsb", bufs=2, name=f"stf{name}")
            nc.scalar.copy(stf, pst)
            sq2 = sb.tile([1, 2], F32, tag="sq2", bufs=2, name=f"sq2{name}")
            nc.vector.reduce_sum(sq2, stf.rearrange("p (g m) -> p g m", g=2),
                                 axis=AX.X)
            ex2 = sb.tile([1, 2], F32, tag="ex2", bufs=2, name=f"ex2{name}")
            nc.scalar.mul(ex2, sq2, 1.0 / ntot)  # (E[x], E[x^2])
            m2 = sb.tile([1, 1], F32, tag="m2", bufs=2, name=f"m2{name}")
            nc.vector.tensor_mul(m2, ex2[:, 0:1], ex2[:, 0:1])
            varv = sb.tile([1, 1], F32, tag="varv", bufs=2, name=f"varv{name}")
            nc.vector.tensor_sub(varv, ex2[:, 1:2], m2)
            # istd = exp(-0.5 * ln(var + eps))  (ACT Rsqrt is banned)
            lv = sb.tile([1, 1], F32, tag="lv", bufs=2, name=f"lv{name}")
            nc.scalar.activation(lv, varv, AF.Ln, bias=eps11, scale=1.0)
            istd = sb.tile([1, 1], F32, tag="istd", bufs=2, name=f"istd{name}")
            nc.scalar.activation(istd, lv, AF.Exp, bias=0.0, scale=-0.5)
            mi = sb.tile([1, 1], F32, tag="mi", bufs=2, name=f"mi{name}")
            nc.vector.tensor_mul(mi, ex2[:, 0:1], istd)
            nmi = sb.tile([1, 1], F32, tag="nmi", bufs=2, name=f"nmi{name}")
            nc.vector.tensor_scalar_mul(nmi, mi, -1.0)
            pair = sb.tile([1, 2], F32, tag="pair", bufs=2, name=f"pair{name}")
            nc.vector.tensor_copy(pair[:, 0:1], istd)
            nc.vector.tensor_copy(pair[:, 1:2], nmi)
            pbc = pp.tile([p_out, 2], F32, tag="psmall", bufs=2, name=f"pbc{name}")
            nc.tensor.matmul(pbc, lhsT=ones_row[:, 0:p_out], rhs=pair,
                             start=True, stop=True)
            bc = sb.tile([p_out, 2], F32, tag="bcast", bufs=2, name=f"bc{name}")
            nc.scalar.copy(bc, pbc)
            return bc

        def conv_layer(s, li, rhs_tiles, KT, w_ap, bias_sb):
            """conv1d(k=3) via matmuls; returns (raw bf16 tiles, stats)."""
            stats = sb.tile([128, 2 * CT], F32, tag="stats", bufs=2,
                            name=f"stats{li}_{s}")
            raws = []
            for mt in range(CT):
                wst = sb.tile([128, KW, KT, 128], BF16, tag="wst", bufs=3,
                              name=f"w{li}_{s}_{mt}")
                for k in range(KW):
                    nc.sync.dma_start(wst[:, k], w_ap[k, mt])
                psc = pp.tile([128, L], F32, tag="pbig", bufs=2,
                              name=f"psc{li}_{s}_{mt}")
                for dt_i in range(KT):
                    for k in range(KW):
                        st = dt_i == 0 and k == 0
                        sp = dt_i == KT - 1 and k == KW - 1
                        for nh in range(NH):
                            nc.tensor.matmul(
                                psc[:, nh * NSPL:(nh + 1) * NSPL],
                                lhsT=wst[:, k, dt_i, :],
                                rhs=rhs_tiles[dt_i][:, nh * NSPL + k:
                                                    nh * NSPL + k + NSPL],
                                start=st, stop=sp)
                hr = sb.tile([128, L], BF16, tag="hraw", bufs=CT,
                             name=f"hr{li}_{s}_{mt}")
                nc.scalar.activation(hr, psc, AF.Identity,
                                     bias=bias_sb[:, mt:mt + 1], scale=1.0,
                                     accum_out=stats[:, mt:mt + 1])
                sqd = sb.tile([128, L], F32, tag="scr", bufs=3,
                              name=f"sq{li}_{s}_{mt}")
                nc.scalar.activation(sqd, hr, AF.Square,
                                     accum_out=stats[:, CT + mt:CT + mt + 1])
                raws.append(hr)
            return raws, stats

        def ln_apply(s, li, raws, bc, gw_ap, gb_ap):
            norms = []
            for mt in range(CT):
                gt = sb.tile([128, L], F32, tag="ln", bufs=4, name=f"g{li}_{s}_{mt}")
                nc.sync.dma_start(gt, gw_ap[mt * 128:(mt + 1) * 128, :])
                bt = sb.tile([128, L], F32, tag="ln", bufs=4, name=f"b{li}_{s}_{mt}")
                nc.sync.dma_start(bt, gb_ap[mt * 128:(mt + 1) * 128, :])
                hh = sb.tile([128, L], F32, tag="scr", bufs=3, name=f"hh{li}_{s}_{mt}")
                nc.scalar.activation(hh, raws[mt], AF.Identity,
                                     bias=bc[:, 1:2], scale=bc[:, 0:1])
                nc.vector.tensor_mul(hh, hh, gt)
                hn = sb.tile([128, L + 2], BF16, tag="hnorm", bufs=CT,
                             name=f"hn{li}_{s}_{mt}")
                # pads via vector engine: same proc as the interior writer so
                # downstream matmuls need a single sem wait for this tile
                nc.vector.memset(hn[:, 0:1], 0.0)
                nc.vector.memset(hn[:, L + 1:L + 2], 0.0)
                nc.vector.tensor_tensor(hn[:, 1:L + 1], hh, bt, op=ALU.add)
                norms.append(hn)
            return norms

        # ================ per-sample pipeline ================
        for s in range(BPC):
            # ---- conv1 input: transposed feats, pre-padded on host
            xts = []
            for dt_i in range(DT):
                xt = sb.tile([128, L + 2], BF16, tag="xt", bufs=DT,
                             name=f"xt{s}_{dt_i}")
                nc.sync.dma_start(xt,
                                  ins["featsT"][s, dt_i * 128:(dt_i + 1) * 128, :])
                xts.append(xt)

            h1r, stats1 = conv_layer(s, 1, xts, DT, ins["w1tp"], b1sb)
            bc1 = ln_finalize(stats1, 128, CT, float(D2 * L), 128, f"1_{s}")
            h1n = ln_apply(s, 1, h1r, bc1, ins["ln1w"], ins["ln1b"])

            h2r, stats2 = conv_layer(s, 2, h1n, CT, ins["w2tp"], b2sb)
            bc2 = ln_finalize(stats2, 128, CT, float(D2 * L), 128, f"2_{s}")
            h2n = ln_apply(s, 2, h2r, bc2, ins["ln2w"], ins["ln2b"])

            # ---- conv3: [2, L] output (2 channels on partitions 0-1)
            stats3 = sb.tile([2, 2 * NH], F32, tag="stats3", bufs=2, name=f"st3_{s}")
            h3 = sb.tile([2, L], F32, tag="row", bufs=4, name=f"h3_{s}")
            for nh in range(NH):
                ps3 = pp.tile([2, NSPL], F32, tag="psmall", bufs=2,
                              name=f"ps3_{s}_{nh}")
                for ct in range(CT):
                    for k in range(KW):
                        nc.tensor.matmul(
                            ps3,
                            lhsT=w3sb[:, ct, k, :],
                            rhs=h2n[ct][:, nh * NSPL + k:nh * NSPL + k + NSPL],
                            start=(ct == 0 and k == 0),
                            stop=(ct == CT - 1 and k == KW - 1))
                nc.scalar.activation(h3[:, nh * NSPL:(nh + 1) * NSPL], ps3,
                                     AF.Identity, bias=b3sb, scale=1.0,
                                     accum_out=stats3[:, nh:nh + 1])
                sq3 = sb.tile([2, NSPL], F32, tag="scr", bufs=3, name=f"sq3_{s}_{nh}")
                nc.scalar.activation(sq3, h3[:, nh * NSPL:(nh + 1) * NSPL],
                                     AF.Square,
                                     accum_out=stats3[:, NH + nh:NH + nh + 1])
            bc3 = ln_finalize(stats3, 2, NH, float(2 * L), 2, f"3_{s}")
            # LN3 apply in place on h3
            nc.scalar.activation(h3, h3, AF.Identity, bias=bc3[:, 1:2],
                                 scale=bc3[:, 0:1])
            g3t = sb.tile([2, L], F32, tag="ln", bufs=4, name=f"g3_{s}")
            nc.sync.dma_start(g3t, ins["ln3w"])
            b3t = sb.tile([2, L], F32, tag="ln", bufs=4, name=f"b3_{s}")
            nc.sync.dma_start(b3t, ins["ln3b"])
            nc.vector.tensor_mul(h3, h3, g3t)
            nc.vector.tensor_add(h3, h3, b3t)

            # ---- softmax pieces (row 0 -> gaussian width, row 1 -> weights)
            mxn = sb.tile([2, 1], F32, tag="mxn", bufs=2, name=f"mxn_{s}")
            nc.vector.reduce_max(mxn, h3, axis=AX.X, negate=True)  # -max
            e3 = sb.tile([2, L], F32, tag="row", bufs=4, name=f"e3_{s}")
            nc.scalar.activation(e3, h3, AF.Exp, bias=mxn, scale=1.0)
            sm = sb.tile([2, 1], F32, tag="sm", bufs=2, name=f"sm_{s}")
            nc.vector.reduce_sum(sm, e3, axis=AX.X)

            # move row-1 scalars (-max1, sum1) to partition 0 via selector matmul
            ms2 = sb.tile([2, 2], F32, tag="ms2", bufs=2, name=f"ms2_{s}")
            nc.vector.tensor_copy(ms2[:, 0:1], mxn)
            nc.vector.tensor_copy(ms2[:, 1:2], sm)
            pms = pp.tile([1, 2], F32, tag="psmall", bufs=2, name=f"pms_{s}")
            nc.tensor.matmul(pms, lhsT=e1col, rhs=ms2, start=True, stop=True)
            ms0 = sb.tile([1, 2], F32, tag="ms0", bufs=2, name=f"ms0_{s}")
            nc.scalar.copy(ms0, pms)
            lnS = sb.tile([1, 1], F32, tag="lnS", bufs=2, name=f"lnS_{s}")
            nc.scalar.activation(lnS, ms0[:, 1:2], AF.Ln, bias=0.0, scale=1.0)
            negoff = sb.tile([1, 1], F32, tag="negoff", bufs=2, name=f"negoff_{s}")
            nc.vector.tensor_sub(negoff, ms0[:, 0:1], lnS)  # -max1 - ln(sum1)

            # ln(w) row at partition 0: select h3 row 1, add offset
            plnw = pp.tile([1, L], F32, tag="pbig", bufs=2, name=f"plnw_{s}")
            for n0, nw in l_splits:
                nc.tensor.matmul(plnw[:, n0:n0 + nw], lhsT=e1col,
                                 rhs=h3[:, n0:n0 + nw], start=True, stop=True)
            lnwrow = sb.tile([1, L], F32, tag="lnwrow", bufs=2, name=f"lnwrow_{s}")
            nc.scalar.activation(lnwrow, plnw, AF.Identity, bias=negoff, scale=1.0)
            # transpose ln(w) row -> per-j-tile columns [128, JT]
            plc = pp.tile([128, JT], F32, tag="psmall", bufs=2, name=f"plc_{s}")
            for jt in range(JT):
                nc.tensor.matmul(plc[:, jt:jt + 1],
                                 lhsT=lnwrow[:, jt * 128:(jt + 1) * 128],
                                 rhs=ones_row[0:1, 0:1], start=True, stop=True)
            lnwc = sb.tile([128, JT], F32, tag="lnwc", bufs=2, name=f"lnwc_{s}")
            nc.scalar.copy(lnwc, plc)

            # row 0: neg_a[i] = -1 / (1e-5 + (2 R^2 L^2 / sum0^2) * e0[i]^2)
            q = sb.tile([1, L], F32, tag="row", bufs=4, name=f"q_{s}")
            nc.vector.tensor_mul(q, e3[0:1, :], e3[0:1, :])
            r0 = sb.tile([1, 1], F32, tag="r0", bufs=2, name=f"r0_{s}")
            nc.vector.reciprocal(r0, sm[0:1, :])
            r2 = sb.tile([1, 1], F32, tag="r2", bufs=2, name=f"r2_{s}")
            nc.vector.tensor_mul(r2, r0, r0)
            fac = sb.tile([1, 1], F32, tag="fac", bufs=2, name=f"fac_{s}")
            nc.scalar.mul(fac, r2, 2.0 * RL * RL)
            den = sb.tile([1, L], F32, tag="row", bufs=4, name=f"den_{s}")
            nc.vector.tensor_scalar(den, q, fac, 1e-5, op0=ALU.mult, op1=ALU.add)
            inv = sb.tile([1, L], F32, tag="row", bufs=4, name=f"inv_{s}")
            nc.vector.reciprocal(inv, den)
            pna = pp.tile([128, L], F32, tag="pbig", bufs=2, name=f"pna_{s}")
            for n0, nw in l_splits:
                nc.tensor.matmul(pna[:, n0:n0 + nw], lhsT=negones_row,
                                 rhs=inv[:, n0:n0 + nw], start=True, stop=True)
            nega = sb.tile([128, L], F32, tag="nega", bufs=2, name=f"nega_{s}")
            nc.scalar.copy(nega, pna)

            # ---- attention weights attT[j, i] = exp(diff2*neg_a + ln w_j)
            feas = []
            for jt in range(JT):
                fe = sb.tile([128, D], BF16, tag="fea", bufs=JT, name=f"fe_{s}_{jt}")
                nc.sync.dma_start(fe, ins["featsb"][s, jt * 128:(jt + 1) * 128, :])
                feas.append(fe)
            atts = []
            for jt in range(JT):
                d2t = sb.tile([128, L], BF16, tag="d2", bufs=3, name=f"d2_{s}_{jt}")
                nc.sync.dma_start(d2t, ins["diff2"][jt * 128:(jt + 1) * 128, :])
                expo = sb.tile([128, L], F32, tag="scr", bufs=3, name=f"ex_{s}_{jt}")
                nc.vector.tensor_mul(expo, d2t, nega)
                at = sb.tile([128, L], BF16, tag="attw", bufs=JT, name=f"at_{s}_{jt}")
                nc.scalar.activation(at, expo, AF.Exp, bias=lnwc[:, jt:jt + 1],
                                     scale=1.0)
                atts.append(at)

            # ---- att @ feats
            for mt in range(JT):
                po = pp.tile([128, D], F32, tag="pbig", bufs=2, name=f"po_{s}_{mt}")
                for jt in range(JT):
                    for n0, nw in att_splits:
                        nc.tensor.matmul(po[:, n0:n0 + nw],
                                         lhsT=atts[jt][:, mt * 128:(mt + 1) * 128],
                                         rhs=feas[jt][:, n0:n0 + nw],
                                         start=(jt == 0), stop=(jt == JT - 1))
                ob = sb.tile([128, D], F32, tag="outsb", bufs=3, name=f"ob_{s}_{mt}")
                nc.vector.tensor_copy(ob, po)
                nc.sync.dma_start(out_ap[s, mt * 128:(mt + 1) * 128, :], ob)


def build_program(BPC, L_=L, D_=D, D2_=D2):
    from concourse import bacc

    nc = bacc.Bacc("TRN2", target_bir_lowering=False, debug=False)
    ins, out_ap = declare_io(nc, BPC, L_, D_, D2_)
    with tile.TileContext(nc) as tc:
        emit_context_pooling(tc, out_ap, ins, BPC=BPC, L=L_, D=D_, D2=D2_)
    nc.compile()
    return nc


def declare_io(nc, BPC, L_, D_, D2_):
    DT, CT = D_ // 128, D2_ // 128

    def mk(name, shape, dt):
        return nc.dram_tensor(name, list(shape), dt, kind="ExternalInput").ap()

    ins = {
        "featsT": mk("featsT", (BPC, D_, L_ + 2), BF16),
        "featsb": mk("featsb", (BPC, L_, D_), BF16),
        "w1tp": mk("w1tp", (KW, CT, 128, DT, 128), BF16),
        "w2tp": mk("w2tp", (KW, CT, 128, CT, 128), BF16),
        "w3p": mk("w3p", (128, CT, KW, 2), BF16),
        "b1p": mk("b1p", (128, CT), F32),
        "b2p": mk("b2p", (128, CT), F32),
        "b3p": mk("b3p", (2, 1), F32),
        "ln1w": mk("ln1w", (D2_, L_), F32),
        "ln1b": mk("ln1b", (D2_, L_), F32),
        "ln2w": mk("ln2w", (D2_, L_), F32),
        "ln2b": mk("ln2b", (D2_, L_), F32),
        "ln3w": mk("ln3w", (2, L_), F32),
        "ln3b": mk("ln3b", (2, L_), F32),
        "diff2": mk("diff2", (L_, L_), BF16),
        "e1p": mk("e1p", (2, 1), F32),
    }
    out_ap = nc.dram_tensor("out", [BPC, L_, D_], F32, kind="ExternalOutput").ap()
    return ins, out_ap


def prep_host(inputs, n_cores, L_=L, D_=D, D2_=D2):
    """Host-side packing: transposes/casts/packs so every device DMA is
    unit-stride. Returns per-core input maps."""
    DT, CT = D_ // 128, D2_ // 128
    feats = np.asarray(inputs["feats"], np.float32)
    Btot = feats.shape[0]
    bpc = Btot // n_cores
    featsT = np.zeros((Btot, D_, L_ + 2), BF)
    featsT[:, :, 1:L_ + 1] = feats.transpose(0, 2, 1).astype(BF)
    featsb = feats.astype(BF)

    def pack_w(w, KT):
        # w [O, I, 3] -> [3, O/128(mt), 128(p over I), I/128(dt), 128(m over O)]
        O, I, _ = w.shape
        a = np.asarray(w, np.float32).transpose(2, 1, 0)           # [3, I, O]
        a = a.reshape(KW, KT, 128, O // 128, 128)                   # [3,dt,p,mt,m]
        a = a.transpose(0, 3, 2, 1, 4)                              # [3,mt,p,dt,m]
        return np.ascontiguousarray(a).astype(BF)

    w1tp = pack_w(np.asarray(inputs["conv1_w"]), DT)
    w2tp = pack_w(np.asarray(inputs["conv2_w"]), CT)
    w3 = np.asarray(inputs["conv3_w"], np.float32)                  # [2, D2, 3]
    w3p = np.ascontiguousarray(
        w3.transpose(1, 2, 0).reshape(CT, 128, KW, 2).transpose(1, 0, 2, 3)
    ).astype(BF)                                                    # [128,ct,k,2]
    b1p = np.ascontiguousarray(
        np.asarray(inputs["conv1_b"], np.float32).reshape(CT, 128).T)
    b2p = np.ascontiguousarray(
        np.asarray(inputs["conv2_b"], np.float32).reshape(CT, 128).T)
    b3p = np.asarray(inputs["conv3_b"], np.float32).reshape(2, 1)
    pos = np.arange(L_, dtype=np.float32)
    diff2 = ((pos[None, :] - pos[:, None]) ** 2).astype(BF)         # [j, i]

    shared = {
        "w1tp": w1tp, "w2tp": w2tp, "w3p": w3p,
        "b1p": b1p, "b2p": b2p, "b3p": b3p,
        "ln1w": np.asarray(inputs["ln1_w"], np.float32),
        "ln1b": np.asarray(inputs["ln1_b"], np.float32),
        "ln2w": np.asarray(inputs["ln2_w"], np.float32),
        "ln2b": np.asarray(inputs["ln2_b"], np.float32),
        "ln3w": np.asarray(inputs["ln3_w"], np.float32),
        "ln3b": np.asarray(inputs["ln3_b"], np.float32),
        "diff2": diff2,
        "e1p": np.array([[0.0], [1.0]], np.float32),
    }
    in_maps = []
    for c in range(n_cores):
        m = dict(shared)
        m["featsT"] = np.ascontiguousarray(featsT[c * bpc:(c + 1) * bpc])
        m["featsb"] = np.ascontiguousarray(featsb[c * bpc:(c + 1) * bpc])
        in_maps.append(m)
    return in_maps, bpc


_PROGRAM_CACHE = {}


def run(inputs, n_cores=N_CORES, trace=False):
    from concourse import bass_utils
    in_maps, bpc = prep_host(inputs, n_cores)
    key = (bpc,)
    if key not in _PROGRAM_CACHE:
        _PROGRAM_CACHE[key] = build_program(bpc)
    nc = _PROGRAM_CACHE[key]
    res = bass_utils.run_bass_kernel_spmd(
        nc, in_maps, core_ids=list(range(n_cores)), trace=trace)
    out = np.concatenate([res.results[c]["out"] for c in range(n_cores)], axis=0)
    return out, res


def kernel(**inputs) -> np.ndarray:
    out, _ = run(inputs)
    return out


# revision 31
# speedup vs baseline: 43.0270x; 43.0270x over previous
"""Trainium2 Bass kernel for nn_ContextPooling (conv stack -> LN -> softmax ->
Gaussian attention pooling). Data-parallel over batch across 8 NeuronCores.

Math notes:
  - conv1d(k=3, same) is computed as 3 shifted matmuls accumulated in PSUM.
  - LayerNorm over (C, L) jointly: stats via activation accum_out (sum) +
    Square pass (sumsq), cross-partition reduction via ones-matmul on PE.
  - gauss / max(gauss, axis=j) == exp(-(i-j)^2 / (1e-5 + 2*std_i^2)) exactly
    (the c/std prefactor is constant along j and cancels).
  - att[i, j] = exp(-(i-j)^2 * a_i) * w_j, with w_j folded in as a per-
    partition bias ln(w_j) inside the ACT Exp.
  - att output: out[i, d] = sum_j attT[j, i] * feats[j, d] -> matmul with
    lhsT = attT tiles (j on partitions), rhs = feats (natural layout).
"""
import numpy as np
import ml_dtypes

import concourse.bass as bass
import concourse.tile as tile
from concourse import mybir

F32 = mybir.dt.float32
BF16 = mybir.dt.bfloat16
AF = mybir.ActivationFunctionType
ALU = mybir.AluOpType
AX = mybir.AxisListType

B, L, D, D2 = 16, 1024, 768, 1536
N_CORES = 8
KW = 3
R = 0.1
BF = ml_dtypes.bfloat16

import os
# tensor_tensor_reduce hard-crashes the device (NRT_EXEC_UNIT_UNRECOVERABLE
# status 101) on this runtime -- keep the ACT Square path by default.
USE_TTR = os.environ.get("K_TTR", "0") == "1"
USE_STT = os.environ.get("K_STT", "1") == "1"  # fused LN scalar chain


def emit_context_pooling(tc, out_ap, ins, *, BPC, L, D, D2, stage="full"):
    nc = tc.nc
    DT, CT, JT = D // 128, D2 // 128, L // 128
    NSPL = min(512, L)
    NH = L // NSPL
    RL = R * L
    att_splits = [(n0, min(512, D - n0)) for n0 in range(0, D, 512)]
    l_splits = [(n0, min(512, L - n0)) for n0 in range(0, L, 512)]

    with (
        tc.tile_pool(name="const", bufs=1) as cpool,
        tc.tile_pool(name="sb", bufs=1) as sb,
        tc.tile_pool(name="ps", bufs=1, space="PSUM") as pp,
    ):
        # ---------------- constants ----------------
        ones_col = cpool.tile([128, 1], F32)
        nc.gpsimd.memset(ones_col, 1.0)
        ones_row = cpool.tile([1, 128], F32)
        nc.gpsimd.memset(ones_row, 1.0)
        negones_row = cpool.tile([1, 128], F32)
        nc.gpsimd.memset(negones_row, -1.0)
        e1col = cpool.tile([2, 1], F32)  # selector (0, 1)^T
        nc.sync.dma_start(e1col, ins["e1p"])
        w3sb = cpool.tile([128, CT, KW, 2], BF16)
        nc.sync.dma_start(w3sb, ins["w3p"])
        b1sb = cpool.tile([128, CT], F32)
        nc.sync.dma_start(b1sb, ins["b1p"])
        b2sb = cpool.tile([128, CT], F32)
        nc.sync.dma_start(b2sb, ins["b2p"])
        b3sb = cpool.tile([2, 1], F32)
        nc.sync.dma_start(b3sb, ins["b3p"])
        eps11 = cpool.tile([1, 1], F32)
        nc.gpsimd.memset(eps11, 1e-5)

        def ln_finalize(stats, p_src, nseg, ntot, p_out, name):
            """stats [p_src, 2*nseg] (sums | sumsqs) -> bc [p_out, 2] =
            (istd, -mean*istd) broadcast to p_out partitions."""
            pst = pp.tile([1, 2 * nseg], F32, tag="psmall", bufs=2, name=f"pst{name}")
            nc.tensor.matmul(pst, lhsT=ones_col[0:p_src, :], rhs=stats,
                             start=True, stop=True)
            stf = sb.tile([1, 2 * nseg], F32, tag="statsb", bufs=2, name=f"stf{name}")
            nc.scalar.mul(stf, pst, 1.0 / ntot)
            ex2 = sb.tile([1, 2], F32, tag="ex2", bufs=2, name=f"ex2{name}")
            nc.vector.reduce_sum(ex2, stf.rearrange("p (g m) -> p g m", g=2),
                                 axis=AX.X)  # (E[x], E[x^2])
            pair = sb.tile([1, 2], F32, tag="pair", bufs=2, name=f"pair{name}")
            lv = sb.tile([1, 1], F32, tag="lv", bufs=2, name=f"lv{name}")
            if USE_STT:
                # nvar = mean^2 - E[x^2] = -var  (one fused DVE op)
                nvar = sb.tile([1, 1], F32, tag="nvar", bufs=2, name=f"nv{name}")
                nc.vector.scalar_tensor_tensor(nvar, ex2[:, 0:1], ex2[:, 0:1],
                                               ex2[:, 1:2], op0=ALU.mult,
                                               op1=ALU.subtract)
                # istd = exp(-0.5 * ln(var + eps))  (ACT Rsqrt is banned)
                nc.scalar.activation(lv, nvar, AF.Ln, bias=eps11, scale=-1.0)
                nc.scalar.activation(pair[:, 0:1], lv, AF.Exp, bias=0.0,
                                     scale=-0.5)
                # pair[1] = -mean * istd
                nc.vector.scalar_tensor_tensor(pair[:, 1:2], ex2[:, 0:1], -1.0,
                                               pair[:, 0:1], op0=ALU.mult,
                                               op1=ALU.mult)
            else:
                m2 = sb.tile([1, 1], F32, tag="m2", bufs=2, name=f"m2{name}")
                nc.vector.tensor_mul(m2, ex2[:, 0:1], ex2[:, 0:1])
                varv = sb.tile([1, 1], F32, tag="varv", bufs=2, name=f"va{name}")
                nc.vector.tensor_sub(varv, ex2[:, 1:2], m2)
                nc.scalar.activation(lv, varv, AF.Ln, bias=eps11, scale=1.0)
                nc.scalar.activation(pair[:, 0:1], lv, AF.Exp, bias=0.0,
                                     scale=-0.5)
                mi = sb.tile([1, 1], F32, tag="mi", bufs=2, name=f"mi{name}")
                nc.vector.tensor_mul(mi, ex2[:, 0:1], pair[:, 0:1])
                nc.vector.tensor_scalar_mul(pair[:, 1:2], mi, -1.0)
            pbc = pp.tile([p_out, 2], F32, tag="psmall", bufs=2, name=f"pbc{name}")
            nc.tensor.matmul(pbc, lhsT=ones_row[:, 0:p_out], rhs=pair,
                             start=True, stop=True)
            bc = sb.tile([p_out, 2], F32, tag="bcast", bufs=2, name=f"bc{name}")
            nc.scalar.copy(bc, pbc)
            return bc

        def conv_layer(s, li, rhs_tiles, KT, w_ap, bias_sb):
            """conv1d(k=3) via matmuls; returns (raw bf16 tiles, stats).
            One 1-bank PSUM tile per (mt, nh) half so conv PSUM slots never
            couple to the attention PSUM rotation."""
            nseg = CT * NH
            stats = sb.tile([128, 2 * nseg], F32, tag="stats", bufs=2,
                            name=f"stats{li}_{s}")
            raws = []
            for mt in range(CT):
                wst = sb.tile([128, KW, KT, 128], BF16, tag="wst", bufs=4,
                              name=f"w{li}_{s}_{mt}")
                for k in range(KW):
                    nc.sync.dma_start(wst[:, k], w_ap[k, mt])
                hr = sb.tile([128, L], BF16, tag="hraw", bufs=CT,
                             name=f"hr{li}_{s}_{mt}")
                for nh in range(NH):
                    psc = pp.tile([128, NSPL], F32, tag="pc", bufs=3,
                                  name=f"psc{li}_{s}_{mt}_{nh}")
                    for dt_i in range(KT):
                        for k in range(KW):
                            nc.tensor.matmul(
                                psc,
                                lhsT=wst[:, k, dt_i, :],
                                rhs=rhs_tiles[dt_i][:, nh * NSPL + k:
                                                    nh * NSPL + k + NSPL],
                                start=(dt_i == 0 and k == 0),
                                stop=(dt_i == KT - 1 and k == KW - 1))
                    seg = mt * NH + nh
                    hslice = hr[:, nh * NSPL:(nh + 1) * NSPL]
                    nc.scalar.activation(hslice, psc, AF.Identity,
                                         bias=bias_sb[:, mt:mt + 1], scale=1.0,
                                         accum_out=stats[:, seg:seg + 1])
                    sqd = sb.tile([128, NSPL], F32, tag="scr", bufs=3,
                                  name=f"sq{li}_{s}_{mt}_{nh}")
                    nc.scalar.activation(
                        sqd, hslice, AF.Square,
                        accum_out=stats[:, nseg + seg:nseg + seg + 1])
                raws.append(hr)
            return raws, stats

        def ln_apply(s, li, raws, bc, gw_ap, gb_ap):
            norms = []
            for mt in range(CT):
                gt = sb.tile([128, L], F32, tag="ln", bufs=4, name=f"g{li}_{s}_{mt}")
                nc.sync.dma_start(gt, gw_ap[mt * 128:(mt + 1) * 128, :])
                bt = sb.tile([128, L], F32, tag="ln", bufs=4, name=f"b{li}_{s}_{mt}")
                nc.sync.dma_start(bt, gb_ap[mt * 128:(mt + 1) * 128, :])
                hh = sb.tile([128, L], F32, tag="scr", bufs=3, name=f"hh{li}_{s}_{mt}")
                nc.scalar.activation(hh, raws[mt], AF.Identity,
                                     bias=bc[:, 1:2], scale=bc[:, 0:1])
                nc.vector.tensor_mul(hh, hh, gt)
                hn = sb.tile([128, L + 2], BF16, tag="hnorm", bufs=CT,
                             name=f"hn{li}_{s}_{mt}")
                # pads via vector engine: same proc as the interior writer so
                # downstream matmuls need a single sem wait for this tile
                nc.vector.memset(hn[:, 0:1], 0.0)
                nc.vector.memset(hn[:, L + 1:L + 2], 0.0)
                nc.vector.tensor_tensor(hn[:, 1:L + 1], hh, bt, op=ALU.add)
                norms.append(hn)
            return norms

        # ================ per-sample pipeline ================
        for s in range(BPC):
            # ---- conv1 input: transposed feats, pre-padded on host
            xts = []
            for dt_i in range(DT):
                xt = sb.tile([128, L + 2], BF16, tag="xt", bufs=DT,
                             name=f"xt{s}_{dt_i}")
                nc.sync.dma_start(xt,
                                  ins["featsT"][s, dt_i * 128:(dt_i + 1) * 128, :])
                xts.append(xt)
            if stage == "loads":
                continue

            h1r, stats1 = conv_layer(s, 1, xts, DT, ins["w1tp"], b1sb)
            bc1 = ln_finalize(stats1, 128, CT * NH, float(D2 * L), 128,
                              f"1_{s}")
            h1n = ln_apply(s, 1, h1r, bc1, ins["ln1w"], ins["ln1b"])
            if stage == "conv1":
                continue

            h2r, stats2 = conv_layer(s, 2, h1n, CT, ins["w2tp"], b2sb)
            bc2 = ln_finalize(stats2, 128, CT * NH, float(D2 * L), 128,
                              f"2_{s}")
            h2n = ln_apply(s, 2, h2r, bc2, ins["ln2w"], ins["ln2b"])
            if stage == "conv2":
                continue

            # ---- conv3: [2, L] output (2 channels on partitions 0-1)
            stats3 = sb.tile([2, 2 * NH], F32, tag="stats3", bufs=2, name=f"st3_{s}")
            h3 = sb.tile([2, L], F32, tag="row", bufs=4, name=f"h3_{s}")
            for nh in range(NH):
                ps3 = pp.tile([2, NSPL], F32, tag="pc", bufs=3,
                              name=f"ps3_{s}_{nh}")
                for ct in range(CT):
                    for k in range(KW):
                        nc.tensor.matmul(
                            ps3,
                            lhsT=w3sb[:, ct, k, :],
                            rhs=h2n[ct][:, nh * NSPL + k:nh * NSPL + k + NSPL],
                            start=(ct == 0 and k == 0),
                            stop=(ct == CT - 1 and k == KW - 1))
                nc.scalar.activation(h3[:, nh * NSPL:(nh + 1) * NSPL], ps3,
                                     AF.Identity, bias=b3sb, scale=1.0,
                                     accum_out=stats3[:, nh:nh + 1])
                sq3 = sb.tile([2, NSPL], F32, tag="scr", bufs=3, name=f"sq3_{s}_{nh}")
                if USE_TTR:
                    nc.vector.tensor_tensor_reduce(
                        sq3, h3[:, nh * NSPL:(nh + 1) * NSPL],
                        h3[:, nh * NSPL:(nh + 1) * NSPL], scale=1.0, scalar=0.0,
                        op0=ALU.mult, op1=ALU.add,
                        accum_out=stats3[:, NH + nh:NH + nh + 1])
                else:
                    nc.scalar.activation(
                        sq3, h3[:, nh * NSPL:(nh + 1) * NSPL], AF.Square,
                        accum_out=stats3[:, NH + nh:NH + nh + 1])
            bc3 = ln_finalize(stats3, 2, NH, float(2 * L), 2, f"3_{s}")
            # LN3 apply in place on h3
            nc.scalar.activation(h3, h3, AF.Identity, bias=bc3[:, 1:2],
                                 scale=bc3[:, 0:1])
            g3t = sb.tile([2, L], F32, tag="ln", bufs=4, name=f"g3_{s}")
            nc.sync.dma_start(g3t, ins["ln3w"])
            b3t = sb.tile([2, L], F32, tag="ln", bufs=4, name=f"b3_{s}")
            nc.sync.dma_start(b3t, ins["ln3b"])
            nc.vector.tensor_mul(h3, h3, g3t)
            nc.vector.tensor_add(h3, h3, b3t)

            # ---- softmax pieces (row 0 -> gaussian width, row 1 -> weights)
            mxn = sb.tile([2, 1], F32, tag="mxn", bufs=2, name=f"mxn_{s}")
            nc.vector.reduce_max(mxn, h3, axis=AX.X, negate=True)  # -max
            e3 = sb.tile([2, L], F32, tag="row", bufs=4, name=f"e3_{s}")
            nc.scalar.activation(e3, h3, AF.Exp, bias=mxn, scale=1.0)
            sm = sb.tile([2, 1], F32, tag="sm", bufs=2, name=f"sm_{s}")
            nc.vector.reduce_sum(sm, e3, axis=AX.X)

            # move row-1 scalars (-max1, sum1) to partition 0 via selector matmul
            ms2 = sb.tile([2, 2], F32, tag="ms2", bufs=2, name=f"ms2_{s}")
            nc.vector.tensor_copy(ms2[:, 0:1], mxn)
            nc.vector.tensor_copy(ms2[:, 1:2], sm)
            pms = pp.tile([1, 2], F32, tag="psmall", bufs=2, name=f"pms_{s}")
            nc.tensor.matmul(pms, lhsT=e1col, rhs=ms2, start=True, stop=True)
            ms0 = sb.tile([1, 2], F32, tag="ms0", bufs=2, name=f"ms0_{s}")
            nc.scalar.copy(ms0, pms)
            lnS = sb.tile([1, 1], F32, tag="lnS", bufs=2, name=f"lnS_{s}")
            nc.scalar.activation(lnS, ms0[:, 1:2], AF.Ln, bias=0.0, scale=1.0)
            negoff = sb.tile([1, 1], F32, tag="negoff", bufs=2, name=f"negoff_{s}")
            nc.vector.tensor_sub(negoff, ms0[:, 0:1], lnS)  # -max1 - ln(sum1)

            # ln(w) row at partition 0: select h3 row 1, add offset
            lnwrow = sb.tile([1, L], F32, tag="lnwrow", bufs=2, name=f"lnwrow_{s}")
            for n0, nw in l_splits:
                plnw = pp.tile([1, nw], F32, tag="pc", bufs=3,
                               name=f"plnw_{s}_{n0}")
                nc.tensor.matmul(plnw, lhsT=e1col, rhs=h3[:, n0:n0 + nw],
                                 start=True, stop=True)
                nc.scalar.activation(lnwrow[:, n0:n0 + nw], plnw, AF.Identity,
                                     bias=negoff, scale=1.0)
            # transpose ln(w) row -> per-j-tile columns [128, JT]
            plc = pp.tile([128, JT], F32, tag="psmall", bufs=2, name=f"plc_{s}")
            for jt in range(JT):
                nc.tensor.matmul(plc[:, jt:jt + 1],
                                 lhsT=lnwrow[:, jt * 128:(jt + 1) * 128],
                                 rhs=ones_row[0:1, 0:1], start=True, stop=True)
            lnwc = sb.tile([128, JT], F32, tag="lnwc", bufs=2, name=f"lnwc_{s}")
            nc.scalar.copy(lnwc, plc)

            # row 0: neg_a[i] = -1 / (1e-5 + (2 R^2 L^2 / sum0^2) * e0[i]^2)
            q = sb.tile([1, L], F32, tag="row", bufs=4, name=f"q_{s}")
            # q = (2 R^2 L^2) * e0^2  (folded constant)
            nc.vector.scalar_tensor_tensor(q, e3[0:1, :], 2.0 * RL * RL,
                                           e3[0:1, :], op0=ALU.mult,
                                           op1=ALU.mult)
            r0 = sb.tile([1, 1], F32, tag="r0", bufs=2, name=f"r0_{s}")
            nc.vector.reciprocal(r0, sm[0:1, :])
            r2 = sb.tile([1, 1], F32, tag="r2", bufs=2, name=f"r2_{s}")
            nc.vector.tensor_mul(r2, r0, r0)
            den = sb.tile([1, L], F32, tag="row", bufs=4, name=f"den_{s}")
            nc.vector.tensor_scalar(den, q, r2, 1e-5, op0=ALU.mult, op1=ALU.add)
            inv = sb.tile([1, L], F32, tag="row", bufs=4, name=f"inv_{s}")
            nc.vector.reciprocal(inv, den)
            nega = sb.tile([128, L], F32, tag="nega", bufs=2, name=f"nega_{s}")
            for n0, nw in l_splits:
                pna = pp.tile([128, nw], F32, tag="pa", bufs=3,
                              name=f"pna_{s}_{n0}")
                nc.tensor.matmul(pna, lhsT=negones_row, rhs=inv[:, n0:n0 + nw],
                                 start=True, stop=True)
                nc.scalar.copy(nega[:, n0:n0 + nw], pna)

            # ---- attention weights attT[j, i] = exp(diff2*neg_a + ln w_j)
            feas = []
            for jt in range(JT):
                fe = sb.tile([128, D], BF16, tag="fea", bufs=JT, name=f"fe_{s}_{jt}")
                nc.sync.dma_start(fe, ins["featsb"][s, jt * 128:(jt + 1) * 128, :])
                feas.append(fe)
            atts = []
            for jt in range(JT):
                d2t = sb.tile([128, L], BF16, tag="d2", bufs=3, name=f"d2_{s}_{jt}")
                nc.sync.dma_start(d2t, ins["diff2"][jt * 128:(jt + 1) * 128, :])
                expo = sb.tile([128, L], F32, tag="scr", bufs=3, name=f"ex_{s}_{jt}")
                nc.vector.tensor_mul(expo, d2t, nega)
                at = sb.tile([128, L], BF16, tag="attw", bufs=JT, name=f"at_{s}_{jt}")
                nc.scalar.activation(at, expo, AF.Exp, bias=lnwc[:, jt:jt + 1],
                                     scale=1.0)
                atts.append(at)

            if stage == "attgen":
                continue

            # ---- att @ feats
            for mt in range(JT):
                ob = sb.tile([128, D], F32, tag="outsb", bufs=3, name=f"ob_{s}_{mt}")
                for n0, nw in att_splits:
                    po = pp.tile([128, nw], F32, tag="pa", bufs=3,
                                 name=f"po_{s}_{mt}_{n0}")
                    for jt in range(JT):
                        nc.tensor.matmul(po,
                                         lhsT=atts[jt][:, mt * 128:(mt + 1) * 128],
                                         rhs=feas[jt][:, n0:n0 + nw],
                                         start=(jt == 0), stop=(jt == JT - 1))
                    nc.vector.tensor_copy(ob[:, n0:n0 + nw], po)
                nc.sync.dma_start(out_ap[s, mt * 128:(mt + 1) * 128, :], ob)


def build_program(BPC, L_=L, D_=D, D2_=D2, stage="full"):
    from concourse import bacc

    nc = bacc.Bacc("TRN2", target_bir_lowering=False, debug=False)
    ins, out_ap = declare_io(nc, BPC, L_, D_, D2_)
    with tile.TileContext(nc) as tc:
        emit_context_pooling(tc, out_ap, ins, BPC=BPC, L=L_, D=D_, D2=D2_,
                             stage=stage)
    nc.compile()
    return nc


def declare_io(nc, BPC, L_, D_, D2_):
    DT, CT = D_ // 128, D2_ // 128

    def mk(name, shape, dt):
        return nc.dram_tensor(name, list(shape), dt, kind="ExternalInput").ap()

    ins = {
        "featsT": mk("featsT", (BPC, D_, L_ + 2), BF16),
        "featsb": mk("featsb", (BPC, L_, D_), BF16),
        "w1tp": mk("w1tp", (KW, CT, 128, DT, 128), BF16),
        "w2tp": mk("w2tp", (KW, CT, 128, CT, 128), BF16),
        "w3p": mk("w3p", (128, CT, KW, 2), BF16),
        "b1p": mk("b1p", (128, CT), F32),
        "b2p": mk("b2p", (128, CT), F32),
        "b3p": mk("b3p", (2, 1), F32),
        "ln1w": mk("ln1w", (D2_, L_), F32),
        "ln1b": mk("ln1b", (D2_, L_), F32),
        "ln2w": mk("ln2w", (D2_, L_), F32),
        "ln2b": mk("ln2b", (D2_, L_), F32),
        "ln3w": mk("ln3w", (2, L_), F32),
        "ln3b": mk("ln3b", (2, L_), F32),
        "diff2": mk("diff2", (L_, L_), BF16),
        "e1p": mk("e1p", (2, 1), F32),
    }
    out_ap = nc.dram_tensor("out", [BPC, L_, D_], F32, kind="ExternalOutput").ap()
    return ins, out_ap


def prep_host(inputs, n_cores, L_=L, D_=D, D2_=D2):
    """Host-side packing: transposes/casts/packs so every device DMA is
    unit-stride. Returns per-core input maps."""
    DT, CT = D_ // 128, D2_ // 128
    feats = np.asarray(inputs["feats"], np.float32)
    Btot = feats.shape[0]
    bpc = Btot // n_cores
    featsT = np.zeros((Btot, D_, L_ + 2), BF)
    featsT[:, :, 1:L_ + 1] = feats.transpose(0, 2, 1).astype(BF)
    featsb = feats.astype(BF)

    def pack_w(w, KT):
        # w [O, I, 3] -> [3, O/128(mt), 128(p over I), I/128(dt), 128(m over O)]
        O, I, _ = w.shape
        a = np.asarray(w, np.float32).transpose(2, 1, 0)           # [3, I, O]
        a = a.reshape(KW, KT, 128, O // 128, 128)                   # [3,dt,p,mt,m]
        a = a.transpose(0, 3, 2, 1, 4)                              # [3,mt,p,dt,m]
        return np.ascontiguousarray(a).astype(BF)

    w1tp = pack_w(np.asarray(inputs["conv1_w"]), DT)
    w2tp = pack_w(np.asarray(inputs["conv2_w"]), CT)
    w3 = np.asarray(inputs["conv3_w"], np.float32)                  # [2, D2, 3]
    w3p = np.ascontiguousarray(
        w3.transpose(1, 2, 0).reshape(CT, 128, KW, 2).transpose(1, 0, 2, 3)
    ).astype(BF)                                                    # [128,ct,k,2]
    b1p = np.ascontiguousarray(
        np.asarray(inputs["conv1_b"], np.float32).reshape(CT, 128).T)
    b2p = np.ascontiguousarray(
        np.asarray(inputs["conv2_b"], np.float32).reshape(CT, 128).T)
    b3p = np.asarray(inputs["conv3_b"], np.float32).reshape(2, 1)
    pos = np.arange(L_, dtype=np.float32)
    diff2 = ((pos[None, :] - pos[:, None]) ** 2).astype(BF)         # [j, i]

    shared = {
        "w1tp": w1tp, "w2tp": w2tp, "w3p": w3p,
        "b1p": b1p, "b2p": b2p, "b3p": b3p,
        "ln1w": np.asarray(inputs["ln1_w"], np.float32),
        "ln1b": np.asarray(inputs["ln1_b"], np.float32),
        "ln2w": np.asarray(inputs["ln2_w"], np.float32),
        "ln2b": np.asarray(inputs["ln2_b"], np.float32),
        "ln3w": np.asarray(inputs["ln3_w"], np.float32),
        "ln3b": np.asarray(inputs["ln3_b"], np.float32),
        "diff2": diff2,
        "e1p": np.array([[0.0], [1.0]], np.float32),
    }
    in_maps = []
    for c in range(n_cores):
        m = dict(shared)
        m["featsT"] = np.ascontiguousarray(featsT[c * bpc:(c + 1) * bpc])
        m["featsb"] = np.ascontiguousarray(featsb[c * bpc:(c + 1) * bpc])
        in_maps.append(m)
    return in_maps, bpc


_EXEC_CACHE = {}


def build_executor(nc, n_cores):
    """jit-compiled SPMD executor for `nc` over `n_cores` devices. Mirrors
    bass2jax.run_bass_via_pjrt but caches the compiled callable so repeat
    calls skip re-lowering/compiling."""
    import jax
    from jax.experimental.shard_map import shard_map
    from jax.sharding import Mesh, PartitionSpec

    from concourse import bass2jax

    bass2jax.install_neuronx_cc_hook()
    partition_name = (
        nc.partition_id_tensor.name if nc.partition_id_tensor else None)
    in_names, out_names, out_avals, zero_outs = [], [], [], []
    for alloc in nc.m.functions[0].allocations:
        if not isinstance(alloc, mybir.MemoryLocationSet):
            continue
        name = alloc.memorylocations[0].name
        if alloc.kind == "ExternalInput":
            if name != partition_name:
                in_names.append(name)
        elif alloc.kind == "ExternalOutput":
            shape = tuple(alloc.tensor_shape)
            dtype = mybir.dt.np(alloc.dtype)
            out_names.append(name)
            out_avals.append(jax.core.ShapedArray(shape, dtype))
            zero_outs.append(np.zeros(shape, dtype))
    n_params = len(in_names)
    all_in_names = list(in_names) + list(out_names)
    if partition_name is not None:
        all_in_names.append(partition_name)

    def _body(*args):
        operands = list(args)
        if partition_name is not None:
            operands.append(bass2jax.partition_id_tensor())
        outs = bass2jax._bass_exec_p.bind(
            *operands,
            out_avals=tuple(out_avals),
            in_names=tuple(all_in_names),
            out_names=tuple(out_names),
            lowering_input_output_aliases=(),
            sim_require_finite=True,
            sim_require_nnan=True,
            nc=nc,
        )
        return tuple(outs)

    devices = jax.devices()[:n_cores]
    mesh = Mesh(np.asarray(devices), ("core",))
    nin = n_params + len(zero_outs)
    fn = jax.jit(
        shard_map(_body, mesh=mesh,
                  in_specs=(PartitionSpec("core"),) * nin,
                  out_specs=(PartitionSpec("core"),) * len(out_names),
                  check_rep=False),
        keep_unused=True,
    )
    sharding = jax.sharding.NamedSharding(mesh, PartitionSpec("core"))
    return fn, in_names, out_names, zero_outs, sharding


def get_executor(bpc, n_cores):
    key = (bpc, n_cores)
    if key not in _EXEC_CACHE:
        nc = build_program(bpc)
        _EXEC_CACHE[key] = (nc, *build_executor(nc, n_cores))
    return _EXEC_CACHE[key]


def run(inputs, n_cores=N_CORES):
    import jax

    in_maps, bpc = prep_host(inputs, n_cores)
    nc, fn, in_names, out_names, zero_outs, sharding = get_executor(
        bpc, n_cores)
    concat_in = [
        np.concatenate([in_maps[c][name] for c in range(n_cores)], axis=0)
        for name in in_names
    ]
    concat_zero = [
        np.zeros((n_cores * z.shape[0], *z.shape[1:]), z.dtype)
        for z in zero_outs
    ]
    args = [jax.device_put(a, sharding) for a in concat_in + concat_zero]
    outs = fn(*args)
    out = np.asarray(outs[out_names.index("out")]).reshape(
        n_cores * bpc, L, D)
    return out


def kernel(**inputs) -> np.ndarray:
    return run(inputs)
